# revision 4
# baseline (speedup 1.0000x reference)
"""BlockDecay (RetNet-style chunkwise linear attention, per-feature decay)
Trainium2 Bass kernel, batch-parallel over 8 NeuronCores.  v21.

All-bf16 compute path, C=256 outer chunks with the exact 3-block causal
structure (even-tri / ext-full / odd-tri), per chunk M:
  PE:  A-blocks (contract r) -> ATb[128,384] PSUM
       SP = k2n_e^T hn_e + k2n_o^T hn_o + diag(g256) @ S_prev   (state)
       OT = hn^T @ Am  (+ S_prev^T @ qsT inter)  -> [d, i] PSUM
  DVE: Am = ATb * tri3 -> bf16      ACT: S egress bf16, otT egress bf16
Host pre-scales with C2=256 phases (bf16):
  qsT = (q gamma^(i%256)).T [R,W], ksT = (k gamma^-(j%256)).T [R,W],
  k2n = blockified k gamma^(256-j%256), hn = blockified h,
  tri3 = triJ|ones|triJ, dg = diag(gamma^256).
Out otT [D, W] bf16 (transposed); host converts to fp32 + transposes.
Input DMA on the sync+gpsimd rings (piece-interleaved), output pieces
alternate rings, issued immediately after each staging copy (a pending
sem-wait parked on gpsimd stalls SWDGE descriptor handling - keep waits
short there).  2-iteration software pipeline; see git-history kernels
v13-v20 for the measured ablations.
"""
import os
import sys
import numpy as np

for _p in ("/root/.axon_site", "/root/.axon_site/_ro/trn_rl_repo",
           "/root/.axon_site/_ro/pypackages"):
    if _p not in sys.path and os.path.isdir(_p):
        sys.path.append(_p)

B, W, R, D = 8, 4096, 128, 128
C2 = 256
NCH = W // C2
NBLK = W // 128

_PROG = {}


def _patched_tc(nc):
    """TileContext with a cheap exit (see v11 notes)."""
    import concourse.tile as tile
    import concourse.tile_sem_assignment as tsa
    from concourse.tile import ScopedClock

    class PatchedTileContext(tile.TileContext):
        def _drain_and_barrier(self, tick_clock, wait_clock):
            gc = tick_clock.global_clock
            n = tsa.N_PROCS
            nc = self.nc
            for p in range(n):
                ticks = gc[p]
                if ticks <= 0:
                    continue
                d = nc.sync.drain()
                wait_clock.add_sem_waits(
                    d.ins,
                    ScopedClock({None: tsa.VectorClock(
                        [ticks if q == p else 0 for q in range(n)])}),
                )
            nc.all_engine_barrier()
            assert self.sems is not None
            popped = nc._tile_sem_poison_stack.pop()
            assert popped is self._sem_poison
            nc.clear_and_free_semaphores(list(self.sems.allocated().values()))

    return PatchedTileContext(nc)


def _split_multi_waits(nc, limit=1):
    import concourse.mybir as mybir
    n_new = 0
    for fn in nc.m.functions:
        for bb in fn.blocks:
            out = []
            changed = False
            for inst in bb.instructions:
                si = getattr(inst, "sync_info", None)
                waits = list(si.on_wait) if si is not None and si.on_wait else []
                if len(waits) > limit:
                    for w in waits[:-limit]:
                        nop = mybir.InstNoOp(
                            name=f"I-wsplit-{n_new}",
                            engine=inst.engine,
                            sync_info=mybir.SyncInfo(on_wait=[w], on_update=[]),
                        )
                        n_new += 1
                        out.append(nop)
                    si.on_wait = waits[-limit:]
                    changed = True
                out.append(inst)
            if changed:
                bb.instructions = out
    return n_new


def _build_program():
    key = "v27f"
    if key in _PROG:
        return _PROG[key]
    import concourse.bass as bass
    import concourse.mybir as mybir

    F32 = mybir.dt.float32
    BF = mybir.dt.bfloat16

    nc = bass.Bass()
    qsT = nc.declare_dram_parameter("qsT", [128, W], BF, isOutput=False)
    ksT = nc.declare_dram_parameter("ksT", [128, W], BF, isOutput=False)
    k2n = nc.declare_dram_parameter("k2n", [128, W], BF, isOutput=False)
    hn = nc.declare_dram_parameter("hn", [128, W], BF, isOutput=False)
    tri3 = nc.declare_dram_parameter("tri3", [128, 384], BF, isOutput=False)
    dg = nc.declare_dram_parameter("dg", [128, 128], BF, isOutput=False)
    otT = nc.declare_dram_parameter("otT", [128, W], BF, isOutput=True)

    mm = nc.tensor.matmul
    with _patched_tc(nc) as tc:
        with tc.tile_pool(name="big", bufs=1) as big, \
             tc.tile_pool(name="small", bufs=1) as small, \
             tc.tile_pool(name="st", bufs=4) as stp, \
             tc.tile_pool(name="amp", bufs=3) as amp, \
             tc.tile_pool(name="ps_at", bufs=3, space="PSUM") as ps_at, \
             tc.tile_pool(name="ps_ot", bufs=3, space="PSUM") as ps_ot, \
             tc.tile_pool(name="ps_s", bufs=2, space="PSUM") as ps_s:

            qsT_sb = big.tile([128, W], BF, tag="qsT")
            ksT_sb = big.tile([128, W], BF, tag="ksT")
            k2n_sb = big.tile([128, W], BF, tag="k2n")
            hn_sb = big.tile([128, W], BF, tag="hn")
            otT_sb = big.tile([128, W], BF, tag="otT")
            tri3_sb = small.tile([128, 384], BF, tag="tri3")
            dg_sb = small.tile([128, 128], BF, tag="dg")

            # PE warm-up during the DMA window (p-state ramp + HAM gate).
            wz = small.tile([128, 256], BF, tag="wz")
            nc.vector.memset(wz[:], 0.0)
            for _ in range(26):
                wp = ps_ot.tile([128, 512], F32, tag="ot")
                mm(wp[:, 0:256], wz[:, :128], wz[:], start=True, stop=True)

            nc.scalar.dma_start(tri3_sb[:], tri3[:])
            nc.scalar.dma_start(dg_sb[:], dg[:])
            P = W // 4
            for p in range(4):
                s = slice(p * P, (p + 1) * P)
                nc.sync.dma_start(ksT_sb[:, s], ksT[:, s])
                nc.gpsimd.dma_start(qsT_sb[:, s], qsT[:, s])
                nc.sync.dma_start(k2n_sb[:, s], k2n[:, s])
                nc.gpsimd.dma_start(hn_sb[:, s], hn[:, s])

            S_prev = stp.tile([128, 128], BF, tag="S")
            nc.vector.memset(S_prev[:], 0.0)

            def at3(M):
                e = slice(M * C2, M * C2 + 128)
                o = slice(M * C2 + 128, M * C2 + 256)
                ATb = ps_at.tile([128, 384], F32, tag="at")
                mm(ATb[:, 0:128], ksT_sb[:, e], qsT_sb[:, e],
                   start=True, stop=True)
                mm(ATb[:, 128:256], ksT_sb[:, e], qsT_sb[:, o],
                   start=True, stop=True)
                mm(ATb[:, 256:384], ksT_sb[:, o], qsT_sb[:, o],
                   start=True, stop=True)
                Am = amp.tile([128, 384], BF, tag="am")
                nc.vector.tensor_mul(Am[:], ATb[:], tri3_sb[:])
                return Am

            def kp_diag(M, S_in):
                e = slice(M * C2, M * C2 + 128)
                o = slice(M * C2 + 128, M * C2 + 256)
                SP = ps_s.tile([128, 128], F32, tag="sp")
                mm(SP[:], k2n_sb[:, e], hn_sb[:, e], start=True, stop=False)
                mm(SP[:], k2n_sb[:, o], hn_sb[:, o], start=False, stop=False)
                mm(SP[:], dg_sb[:], S_in[:], start=False, stop=True)
                S_new = stp.tile([128, 128], BF, tag="S")
                nc.scalar.copy(S_new[:], SP[:])
                return S_new

            def emit_ot(M, Am, S_in, OT, half):
                e = slice(M * C2, M * C2 + 128)
                o = slice(M * C2 + 128, M * C2 + 256)
                c0 = slice(half * 256, half * 256 + 128)
                c1 = slice(half * 256 + 128, half * 256 + 256)
                mm(OT[:, c0], hn_sb[:, e], Am[:, 0:128], start=True, stop=False)
                mm(OT[:, c0], S_in[:], qsT_sb[:, e], start=False, stop=True)
                mm(OT[:, c1], hn_sb[:, e], Am[:, 128:256],
                   start=True, stop=False)
                mm(OT[:, c1], hn_sb[:, o], Am[:, 256:384],
                   start=False, stop=False)
                mm(OT[:, c1], S_in[:], qsT_sb[:, o], start=False, stop=True)

            # software pipeline (2-iter lag): iter M runs OT(M) with Am(M)
            # (masked by DVE ~2 iters earlier) and S_sb from 2 iters back,
            # then AT3(M+2) and KP2+diag(M+1); ACT: S-copy then otT-copy.
            Am = {0: at3(0), 1: at3(1)}
            S_hist = [S_prev, kp_diag(0, S_prev)]  # S_hist[m+1]: state after m
            OT = None
            pend_out = None
            for M in range(NCH):
                if M % 2 == 0:
                    OT = ps_ot.tile([128, 512], F32, tag="ot")
                emit_ot(M, Am.pop(M), S_hist[M], OT, M % 2)
                if M + 2 < NCH:
                    Am[M + 2] = at3(M + 2)
                if M + 1 < NCH:
                    S_hist.append(kp_diag(M + 1, S_hist[M + 1]))
                if pend_out is not None:
                    sl, pOT, on_sync = pend_out
                    nc.scalar.copy(otT_sb[:, sl], pOT[:])
                    if on_sync:
                        nc.sync.dma_start(otT[:, sl], otT_sb[:, sl])
                    else:
                        nc.gpsimd.dma_start(otT[:, sl], otT_sb[:, sl])
                    pend_out = None
                if M % 2 == 1:
                    pend_out = (slice((M - 1) * C2, (M + 1) * C2), OT,
                                (M // 2) % 2 == 1)
            sl, pOT, on_sync = pend_out
            nc.scalar.copy(otT_sb[:, sl], pOT[:])
            nc.sync.dma_start(otT[:, sl], otT_sb[:, sl])

    _split_multi_waits(nc)
    _PROG[key] = nc
    return nc


def _host_prep(q_alpha, k, h_norm, gamma_vec, causal_mask):
    gamma = np.clip(np.asarray(gamma_vec, np.float64), 1e-8, None)
    log_g = np.log(gamma)
    i_loc = (np.arange(W) % C2).astype(np.float64)
    Sq = np.exp(np.outer(i_loc, log_g))          # [W, R] gamma^(i%256)
    Skneg = np.exp(np.outer(-i_loc, log_g))      # gamma^-(j%256)
    Sk2 = np.exp(np.outer(C2 - i_loc, log_g))    # gamma^(256 - j%256)
    dgv = np.exp(C2 * log_g)

    import ml_dtypes
    BFD = ml_dtypes.bfloat16
    triJ = np.ascontiguousarray(np.asarray(causal_mask, np.float32).T)
    tri3 = np.concatenate([triJ, np.ones((128, 128), np.float32), triJ],
                          axis=1).astype(BFD)
    dgm = np.ascontiguousarray(np.diag(dgv).astype(BFD))

    def blockify(x):  # [W, 128] -> [128, (blk, 128)]
        return np.ascontiguousarray(
            x.reshape(NBLK, 128, 128).transpose(1, 0, 2).reshape(128, W))

    in_maps = []
    for b in range(B):
        q64 = np.asarray(q_alpha[b], np.float64)
        k64 = np.asarray(k[b], np.float64)
        in_maps.append({
            "qsT": np.ascontiguousarray((q64 * Sq).T.astype(BFD)),
            "ksT": np.ascontiguousarray((k64 * Skneg).T.astype(BFD)),
            "k2n": blockify((k64 * Sk2).astype(BFD)),
            "hn": blockify(np.ascontiguousarray(
                np.asarray(h_norm[b], BFD))),
            "tri3": tri3,
            "dg": dgm,
        })
    return in_maps


def _ensure_ntff_hook():
    try:
        from antenv import axon_hooks  # noqa: F401
        return
    except ImportError:
        pass
    import types
    import antenv
    try:
        import trn_agent_boot.trn_boot as tb
        hook = tb._ntff_profile_via_ctypes("/opt/axon/libaxon_pjrt.so")
    except Exception:
        hook = None
    mod = types.ModuleType("antenv.axon_hooks")
    mod.get_axon_ntff_profile_hook = lambda: hook
    mod.set_axon_ntff_profile_hook = lambda h: None
    sys.modules["antenv.axon_hooks"] = mod
    antenv.axon_hooks = mod


_last = {"exec_time_ns": None}


def kernel(q_alpha, k, h_norm, gamma_vec, causal_mask, decay_diff,
           _trace=False):
    trace = _trace or os.environ.get("BD_TRACE", "0") == "1"
    from concourse.bass_utils import run_bass_kernel_spmd

    nc = _build_program()
    in_maps = _host_prep(q_alpha, k, h_norm, gamma_vec, causal_mask)
    kwargs = {}
    if trace:
        _ensure_ntff_hook()
        import concourse.bass_utils as bu
        bu.upload_artifacts = lambda tmpdir: tmpdir
        kwargs = dict(trace=True, tmpdir=os.environ.get("BD_TRACE_DIR") or None)
    res = run_bass_kernel_spmd(nc, in_maps, list(range(B)), **kwargs)
    _last["exec_time_ns"] = res.exec_time_ns
    out = np.empty((B, W, D), np.float32)
    for b in range(B):
        out[b] = np.asarray(res.results[b]["otT"]).astype(np.float32).T
    return out
